# revision 13
# baseline (speedup 1.0000x reference)
"""Multi-head attention (S=2048, B=2, D=1024, H=16) on 8 TRN2 NeuronCores.

Sharding: batch*heads across cores — core c owns heads {2c, 2c+1} for both
batches (4 (head, batch) pairs per core, d_k=64 each -> a 128-row slice of
every projection). The output projection is row-parallel after an AllToAll
that redistributes per-head context to per-sequence-block context.

v2 schedule (single NEFF per core), built around the ACT-exp roofline
(~147us for 128 x [128,1024] Exp instrs per core):
  - Batch-0-first streaming: k0,v0,q0 x-tiles stream on SP, projections
    trail each tensor, V0 transposed to V_aug via PE-transpose (prefix PE
    is idle; SP transposes would gate the stream). Attention b0 starts
    ~40us in and the scalar engine stays saturated from there on.
  - b1 x streams during b0 attention as [128,1024] half-tiles; b1
    projections run as PE bursts accumulating into *borrowed* sps-pool
    PSUM tiles woven between score/ctx matmuls (PE has ~0.5us/jt slack
    vs ACT). V1 uses DMA-transposes on the then-idle SP queue.
  - Flash-style attention per (head, batch) in S^T orientation as in v1:
    S^T = K^T_blk.T @ Q^T (K=64), expS = ACT Exp(0.125 S^T) -> bf16,
    ctx^T[d|Z, i] += V_aug.T @ expS (M=65 carries the Z row).
  - Normalization without DRAM bounces: 1/Z via DVE reciprocal on the
    [1,1024] Z rows straight out of PSUM, partition-broadcast with K=1
    ones-matmuls into a borrowed PSUM tile, one DVE multiply, stage to
    a2a_in. a2a b0 fires under b1 attention; osrc/oproj b0 also run as
    woven borrows under b1 attention. Tail = norm ch3 + a2a b1 + oproj b1.
"""

import numpy as np
import ml_dtypes

import concourse.bass as bass
import concourse.mybir as mybir
import concourse.tile as tile
from concourse import bacc
from concourse.bass_utils import run_bass_kernel_spmd

S = 2048
B = 2
D = 1024
H = 16
DK = 64
N_CORES = 8
SCALE = 1.0 / np.sqrt(DK)

F32 = mybir.dt.float32
BF16 = mybir.dt.bfloat16

SB = S * B                      # 4096 total cols (i = b*S + s)
ROWS_PER_CORE = SB // N_CORES   # 512 output rows per core
SEQ_PER_CORE = S // N_CORES     # 256

_cached = {}


def build_program():
    if "nc" in _cached:
        return _cached["nc"]
    nc = bacc.Bacc("TRN2", target_bir_lowering=False, debug=False,
                   num_devices=N_CORES)

    xT = {t: nc.dram_tensor(f"x{t}T", [D, SB], BF16, kind="ExternalInput")
          for t in "qkv"}
    wT = {t: nc.dram_tensor(f"w{t}T", [D, 128], BF16, kind="ExternalInput")
          for t in "qkv"}
    bvec = {t: nc.dram_tensor(f"b{t}", [128, 1], F32, kind="ExternalInput")
            for t in "qkv"}
    woT = nc.dram_tensor("woT", [D, D], BF16, kind="ExternalInput")
    bo_bc = nc.dram_tensor("bo_bc", [128, D], F32, kind="ExternalInput")
    ident = nc.dram_tensor("ident", [128, 128], BF16, kind="ExternalInput")
    out_d = nc.dram_tensor("out", [ROWS_PER_CORE, D], F32, kind="ExternalOutput")

    a2a_in = [nc.dram_tensor(f"a2a_in{b}", [N_CORES * 128, SEQ_PER_CORE], BF16)
              for b in range(B)]
    a2a_out = [nc.dram_tensor(f"a2a_out{b}", [N_CORES * 128, SEQ_PER_CORE], BF16)
               for b in range(B)]

    with tile.TileContext(nc) as tc:
        _emit(nc, tc, xT, wT, bvec, woT, bo_bc, ident, out_d, a2a_in, a2a_out)
    nc.compile()
    _cached["nc"] = nc
    return nc


def _emit(nc, tc, xT, wT, bvec, woT, bo_bc, ident, out_d, a2a_in, a2a_out):
    from contextlib import ExitStack

    ICH = 1024          # i-chunk width (one chunk = half a batch)
    NMM = 512           # max free dim per matmul into one PSUM bank
    JT = S // 128       # 16 j-tiles per (head, batch) pair
    VA = 128            # V_aug block stride
    NCH = 4             # chunks: (b, half) = ch//2, ch%2

    with ExitStack() as top:
        const = top.enter_context(tc.tile_pool(name="const", bufs=1))
        w_sb = const.tile([128, 3 * 8 * 128], BF16)
        bias_sb = const.tile([128, 3], F32)
        ident_sb = const.tile([128, 128], BF16)
        ones_sb = const.tile([1, 64], BF16)
        qT_sb = [const.tile([128, S], BF16, name=f"qT{b}") for b in range(B)]
        kT_sb = [const.tile([128, S], BF16, name=f"kT{b}") for b in range(B)]
        vT_sb = [const.tile([128, S], BF16, name=f"vT{b}") for b in range(B)]
        vaug_sb = const.tile([128, 4 * JT * VA], BF16)
        bo_sb = const.tile([128, D], F32)
        wo_sb = const.tile([128, 2 * N_CORES * NMM], BF16)
        osrc_sb = const.tile([128, B * N_CORES * SEQ_PER_CORE], BF16)
        warm_sb = const.tile([1, 8], F32)

        nc.vector.memset(vaug_sb[:], 1.0)
        nc.vector.memset(ones_sb[:], 1.0)
        nc.vector.memset(warm_sb[:], 0.0)
        # load the Exp table set before attention needs it
        nc.scalar.activation(warm_sb[:], warm_sb[:],
                             mybir.ActivationFunctionType.Exp, scale=1.0)

        x0pool = top.enter_context(tc.tile_pool(name="x0", bufs=6))
        x1pool = top.enter_context(tc.tile_pool(name="x1", bufs=10))
        epool = top.enter_context(tc.tile_pool(name="expS", bufs=8))
        zpool = top.enter_context(tc.tile_pool(name="zrec", bufs=4))
        cupool = top.enter_context(tc.tile_pool(name="ctxu", bufs=2))
        cnpool = top.enter_context(tc.tile_pool(name="ctxn", bufs=2))
        outpool = top.enter_context(tc.tile_pool(name="oout", bufs=3))
        pools = {}

        proj_targets = {"q": qT_sb, "k": kT_sb, "v": vT_sb}
        x0tiles = {}
        x1tiles = {}

        # ---- SP-queue prefix: weights, then b0 x-stream per tensor ----
        def emit_w_loads():
            for t in "kvq":
                ti = "qkv".index(t)
                for kt in range(8):
                    nc.sync.dma_start(
                        w_sb[:, (ti * 8 + kt) * 128:(ti * 8 + kt + 1) * 128],
                        wT[t].ap()[kt * 128:(kt + 1) * 128, :])
                nc.sync.dma_start(bias_sb[:, ti:ti + 1], bvec[t].ap())
            nc.sync.dma_start(ident_sb[:], ident.ap())

        def emit_x0_loads(t):
            for kt in range(8):
                xt = x0pool.tile([128, S], BF16, tag="xs0",
                                 name=f"x0_{t}_{kt}")
                nc.sync.dma_start(
                    xt[:], xT[t].ap()[kt * 128:(kt + 1) * 128, 0:S])
                x0tiles[(t, kt)] = xt

        def emit_x1_loads(t):
            for kt in range(8):
                xt = x1pool.tile([128, S], BF16, tag="xs1",
                                 name=f"x1_{t}_{kt}")
                nc.sync.dma_start(
                    xt[:], xT[t].ap()[kt * 128:(kt + 1) * 128, S:2 * S])
                x1tiles[(t, kt)] = xt

        def emit_proj_b0(t):
            # both column-halves of batch 0, kt-outer, 2 live accumulators
            ti = "qkv".index(t)
            pss = [pools["pp"].tile([128, ICH], F32, tag="pp",
                                    name=f"pp_{t}_{c}") for c in range(2)]
            for kt in range(8):
                xt = x0tiles[(t, kt)]
                for c in range(2):
                    for nn in range(2):
                        nc.tensor.matmul(
                            pss[c][:, nn * NMM:(nn + 1) * NMM],
                            w_sb[:, (ti * 8 + kt) * 128:(ti * 8 + kt + 1) * 128],
                            xt[:, c * ICH + nn * NMM:c * ICH + (nn + 1) * NMM],
                            start=(kt == 0), stop=(kt == 7))
            for c in range(2):
                nc.vector.tensor_scalar_add(
                    proj_targets[t][0][:, c * ICH:(c + 1) * ICH],
                    pss[c][:], bias_sb[:, ti:ti + 1])

        def emit_proj_b1_burst(t, h):
            # one [128,1024] accumulator borrowed from the sps pool
            ti = "qkv".index(t)
            acc = pools["sp"].tile([128, ICH], F32, tag="sp",
                                   name=f"p1_{t}{h}")
            for kt in range(8):
                xt = x1tiles[(t, kt)]
                for nn in range(2):
                    nc.tensor.matmul(
                        acc[:, nn * NMM:(nn + 1) * NMM],
                        w_sb[:, (ti * 8 + kt) * 128:(ti * 8 + kt + 1) * 128],
                        xt[:, h * ICH + nn * NMM:h * ICH + (nn + 1) * NMM],
                        start=(kt == 0), stop=(kt == 7))
            nc.vector.tensor_scalar_add(
                proj_targets[t][1][:, h * ICH:(h + 1) * ICH],
                acc[:], bias_sb[:, ti:ti + 1])

        def emit_vaug_pe(b):
            # PE-transpose vT blocks into V_aug (prefix only, b=0)
            for g in range(4):          # groups of 4 j-tiles
                tp = pools["tp"].tile([128, 512], BF16, tag="tp",
                                      name=f"tp{b}_{g}")
                for k in range(4):
                    jt = g * 4 + k
                    nc.tensor.transpose(
                        tp[:, k * 128:(k + 1) * 128],
                        vT_sb[b][:, jt * 128:(jt + 1) * 128], ident_sb[:])
                for k in range(4):
                    jt = g * 4 + k
                    for hh in range(2):
                        p = hh * 2 + b
                        col = (p * JT + jt) * VA
                        nc.vector.tensor_copy(
                            vaug_sb[:, col:col + 64],
                            tp[:, k * 128 + hh * 64:k * 128 + hh * 64 + 64])

        def emit_vaug_dma(b):
            # DMA-transposes on SP (post-stream idle); b=1
            for jt in range(JT):
                for hh in range(2):
                    p = hh * 2 + b
                    col = (p * JT + jt) * VA
                    nc.sync.dma_start_transpose(
                        vaug_sb[:, col:col + 64],
                        vT_sb[b][hh * 64:hh * 64 + 64,
                                 jt * 128:(jt + 1) * 128])

        def emit_wo_loads():
            for ce in range(D // NMM):
                for s in range(N_CORES):
                    nc.sync.dma_start(
                        wo_sb[:, (ce * N_CORES + s) * NMM:
                              (ce * N_CORES + s + 1) * NMM],
                        woT.ap()[s * 128:(s + 1) * 128, ce * NMM:(ce + 1) * NMM])
            nc.sync.dma_start(bo_sb[:], bo_bc.ap())

        # ---- attention chunk pieces ----
        def emit_attn_jt(ch, jt, cps):
            b, ch2 = ch // 2, ch % 2
            ioff = ch2 * ICH
            for hh in range(2):
                p = hh * 2 + b
                sps = pools["sp"].tile([128, ICH], F32, tag="sp",
                                       name=f"sp{ch}_{jt}_{hh}")
                for nn in range(2):
                    nc.tensor.matmul(
                        sps[:, nn * NMM:(nn + 1) * NMM],
                        kT_sb[b][hh * 64:hh * 64 + 64,
                                 jt * 128:(jt + 1) * 128],
                        qT_sb[b][hh * 64:hh * 64 + 64,
                                 ioff + nn * NMM:ioff + (nn + 1) * NMM],
                        start=True, stop=True)
                es = epool.tile([128, ICH], BF16, tag="es",
                                name=f"es{ch}_{jt}_{hh}")
                nc.scalar.activation(
                    es[:], sps[:], mybir.ActivationFunctionType.Exp,
                    scale=float(SCALE))
                col = (p * JT + jt) * VA
                for nn in range(2):
                    nc.tensor.matmul(
                        cps[hh][:, nn * NMM:(nn + 1) * NMM],
                        vaug_sb[:, col:col + 65],
                        es[:, nn * NMM:(nn + 1) * NMM],
                        start=(jt == 0), stop=(jt == JT - 1))

        def emit_norm_stage(ch, cps):
            # 1/Z from the PSUM Z rows, K=1 ones-matmul broadcast into a
            # borrowed PSUM tile, one multiply, stage into a2a_in.
            b, ch2 = ch // 2, ch % 2
            cu = cupool.tile([128, ICH], F32, tag="cu", name=f"cu{ch}")
            zr = [zpool.tile([1, ICH], BF16, tag="zr", name=f"zr{ch}_{hh}")
                  for hh in range(2)]
            for hh in range(2):
                # Z row: aligned DVE copy out of PSUM, DMA does the
                # 64 -> 0 row placement, then reciprocal at partition 0
                # (reciprocal_approx_* misreads any other base).
                zf = zpool.tile([65, ICH], F32, tag="zf", bufs=2,
                                name=f"zf{ch}_{hh}")
                zw = zpool.tile([1, ICH], F32, tag="zw", bufs=2,
                                name=f"zw{ch}_{hh}")
                zv = zpool.tile([1, ICH], F32, tag="zv", bufs=2,
                                name=f"zv{ch}_{hh}")
                nc.vector.tensor_copy(cu[hh * 64:(hh + 1) * 64, :],
                                      cps[hh][0:64, :])
                nc.vector.tensor_copy(zf[64:65, :], cps[hh][64:65, :])
                nc.sync.dma_start(zw[:], zf[64:65, :])
                nc.vector.reciprocal_approx_fast(zv[:], zw[:])
                nc.vector.tensor_copy(zr[hh][:], zv[:])
            zb = pools["sp"].tile([128, ICH], F32, tag="sp", name=f"zb{ch}")
            for hh in range(2):
                for nn in range(2):
                    nc.tensor.matmul(
                        zb[hh * 64:(hh + 1) * 64, nn * NMM:(nn + 1) * NMM],
                        ones_sb[:], zr[hh][:, nn * NMM:(nn + 1) * NMM],
                        start=True, stop=True)
            cn = cnpool.tile([128, ICH], BF16, tag="cn", name=f"cn{ch}")
            # PSUM operand must be in0 (in1-from-PSUM reads wrong memory)
            nc.vector.tensor_mul(cn[:], zb[:], cu[:])
            for dl in range(ICH // SEQ_PER_CORE):
                d = ch2 * (ICH // SEQ_PER_CORE) + dl
                nc.sync.dma_start(
                    a2a_in[b].ap()[d * 128:(d + 1) * 128, :],
                    cn[:, dl * SEQ_PER_CORE:(dl + 1) * SEQ_PER_CORE])

        def emit_cc(b):
            nc.gpsimd.collective_compute(
                "AllToAll", mybir.AluOpType.bypass,
                replica_groups=[list(range(N_CORES))],
                ins=[a2a_in[b].ap().opt()], outs=[a2a_out[b].ap().opt()])

        def emit_osrc(b):
            for s in range(N_CORES):
                nc.sync.dma_start(
                    osrc_sb[:, (b * N_CORES + s) * SEQ_PER_CORE:
                            (b * N_CORES + s + 1) * SEQ_PER_CORE],
                    a2a_out[b].ap()[s * 128:(s + 1) * 128, :])

        def emit_oproj_ce(b, ce):
            # one borrowed [128,1024] PSUM tile = both 128-row i-blocks
            acc = pools["sp"].tile([128, ICH], F32, tag="sp",
                                   name=f"op{b}_{ce}")
            for s in range(N_CORES):
                wo_t = wo_sb[:, (ce * N_CORES + s) * NMM:
                             (ce * N_CORES + s + 1) * NMM]
                for it in range(2):
                    nc.tensor.matmul(
                        acc[:, it * NMM:(it + 1) * NMM],
                        osrc_sb[:, (b * N_CORES + s) * SEQ_PER_CORE +
                                it * 128:
                                (b * N_CORES + s) * SEQ_PER_CORE +
                                (it + 1) * 128],
                        wo_t, start=(s == 0), stop=(s == N_CORES - 1))
            for it in range(2):
                ot = outpool.tile([128, NMM], F32, tag="ot",
                                  name=f"ot{b}_{ce}_{it}")
                nc.vector.tensor_add(ot[:], acc[:, it * NMM:(it + 1) * NMM],
                                     bo_sb[:, ce * NMM:(ce + 1) * NMM])
                nc.sync.dma_start(
                    out_d.ap()[b * SEQ_PER_CORE + it * 128:
                               b * SEQ_PER_CORE + (it + 1) * 128,
                               ce * NMM:(ce + 1) * NMM], ot[:])

        # ================= prefix =================
        emit_w_loads()
        with tc.tile_pool(name="proj_psum", bufs=2, space="PSUM") as pp, \
             tc.tile_pool(name="tpsum", bufs=2, space="PSUM") as tp:
            pools["pp"], pools["tp"] = pp, tp
            emit_x0_loads("k")
            emit_proj_b0("k")
            emit_x0_loads("v")
            emit_proj_b0("v")
            emit_vaug_pe(0)
            emit_x0_loads("q")
            emit_proj_b0("q")
            # b1 stream + remaining SP work, all queued behind b0
            for t in "kvq":
                emit_x1_loads(t)
            emit_wo_loads()

        # ================= attention =================
        # weave table: (ch, jt) -> list of emit thunks
        weave = {
            (0, 4): [lambda: emit_proj_b1_burst("k", 0)],
            (0, 8): [lambda: emit_proj_b1_burst("k", 1)],
            (0, 12): [lambda: emit_proj_b1_burst("v", 0)],
            (1, 0): [lambda: emit_proj_b1_burst("v", 1)],
            (1, 4): [lambda: emit_proj_b1_burst("q", 0),
                     lambda: emit_vaug_dma(1)],
            (1, 8): [lambda: emit_proj_b1_burst("q", 1)],
        }

        with tc.tile_pool(name="spsum", bufs=2, space="PSUM") as sp, \
             tc.tile_pool(name="cpsum", bufs=2, space="PSUM") as cp:
            pools["sp"], pools["cp"] = sp, cp
            for ch in range(NCH):
                cps = [pools["cp"].tile([65, ICH], F32, tag="cp",
                                        name=f"cp{ch}_{i}") for i in range(2)]
                for jt in range(JT):
                    emit_attn_jt(ch, jt, cps)
                    for thunk in weave.get((ch, jt), ()):
                        thunk()
                    if ch == 3 and jt == 4:
                        emit_osrc(0)
                    if ch == 3 and jt == 6:
                        emit_oproj_ce(0, 0)
                    if ch == 3 and jt == 10:
                        emit_oproj_ce(0, 1)
                emit_norm_stage(ch, cps)
                if ch == 1:
                    emit_cc(0)
            emit_cc(1)
            emit_osrc(1)
            emit_oproj_ce(1, 0)
            emit_oproj_ce(1, 1)


def shard_inputs(inputs):
    q, k, v = inputs["query"], inputs["key"], inputs["value"]
    xt = {}
    for t, x in (("q", q), ("k", k), ("v", v)):
        xt[t] = np.ascontiguousarray(
            np.asarray(x, np.float32).transpose(2, 1, 0).reshape(D, SB)
        ).astype(ml_dtypes.bfloat16)
    woT = np.ascontiguousarray(
        np.asarray(inputs["w_o"], np.float32).T).astype(ml_dtypes.bfloat16)
    bo_bc = np.ascontiguousarray(
        np.tile(np.asarray(inputs["b_o"], np.float32).reshape(1, D), (128, 1)))
    ident = np.eye(128, dtype=ml_dtypes.bfloat16)
    in_maps = []
    for c in range(N_CORES):
        m = {"woT": woT, "bo_bc": bo_bc, "ident": ident}
        for t in "qkv":
            m[f"x{t}T"] = xt[t]
            w = np.asarray(inputs[f"w_{t}"], np.float32)
            bb = np.asarray(inputs[f"b_{t}"], np.float32)
            m[f"w{t}T"] = np.ascontiguousarray(
                w[c * 128:(c + 1) * 128, :].T).astype(ml_dtypes.bfloat16)
            m[f"b{t}"] = np.ascontiguousarray(
                bb[c * 128:(c + 1) * 128].reshape(128, 1))
        in_maps.append(m)
    return in_maps


def unshard(results):
    out = np.empty((S, B, D), np.float32)
    for c in range(N_CORES):
        o = results[c]["out"]          # [512, 1024], row r = b*256 + rr
        for b in range(B):
            out[c * SEQ_PER_CORE:(c + 1) * SEQ_PER_CORE, b, :] = \
                o[b * SEQ_PER_CORE:(b + 1) * SEQ_PER_CORE, :]
    return out


def run(inputs, trace=False, trace_cores=None):
    nc = build_program()
    in_maps = shard_inputs(inputs)
    res = run_bass_kernel_spmd(nc, in_maps, core_ids=list(range(N_CORES)),
                               trace=trace, trace_cores=trace_cores)
    return unshard(res.results), res


def kernel(**inputs):
    out, _ = run(inputs, trace=False)
    return out
